# revision 25
# baseline (speedup 1.0000x reference)
"""Trainium2 Bass kernel for nn_Attention_3633542333119 (additive attention).

reference:
    q_proj   = q @ Wq.T                          # [B, H]
    ref_proj = (ref @ Wref.T).reshape(B, S, H)   # [B, S, H]
    u        = einsum("bsh,h->bs", tanh(q_proj[:, None, :] + ref_proj), v)
    return (u, ref_proj)

B=128, S=2048, H=512. Data-parallel over batch across 8 NeuronCores
(16 batches / 32768 ref rows per core); Wref/v replicated; q_proj is
precomputed on host (tiny: 128x512 @ 512x512) and sharded with the batch.

Device-side plan (per core), all f32, matmuls in fp32r (full-rate):
  - host pre-transposes ref into supertiles reft[g] = ref[g*512:(g+1)*512].T
    (shape [512h, 512n]) so the PE's stationary operand (lhsT, [K=h, M=n])
    loads with plain DMA - no on-device transpose needed.
  - per supertile: psum[mb] = sum_kb reft_blk(kb,mb).T @ wrefT_blk(kb)
    (4 accumulating matmuls, N=512) -> ref_proj tile [128n, 512h'].
  - epilogue: ACT copies psum->SBUF (ref_proj out), DVE adds broadcast
    q_proj row, ACT tanh, DVE fused (tanh*v)+reduce -> u column.
  - u columns staged per batch [128, 16], PE-transposed, DMA'd out.
"""

import numpy as np
from contextlib import ExitStack

B = 128
S = 2048
H = 512
KB = H // 128                   # 128-row chunks of the hidden dim
N_CORES = 8
B_LOC = B // N_CORES            # batches per core
ST_ROWS = 512                   # rows (n) per supertile
ST_PER_BATCH = S // ST_ROWS     # supertiles per batch
N_ST = B_LOC * ST_PER_BATCH     # supertiles per core
ROWS = B_LOC * S                # ref rows per core

_PROGRAM_CACHE = {}
LAST_RESULT = None              # BassKernelResults of the last kernel() call


def _build_program(b_loc=B_LOC, st_per_batch=ST_PER_BATCH):
    import concourse.bass as bass
    import concourse.tile as tile
    import concourse.mybir as mybir
    from concourse import bacc

    f32 = mybir.dt.float32
    f32r = mybir.dt.float32r
    KB = H // 128               # contraction (h) chunks (4)
    CB = H // 128               # output (h') chunks per supertile (4)
    n_st = b_loc * st_per_batch
    s_loc = st_per_batch * ST_ROWS

    nc = bacc.Bacc("TRN2", target_bir_lowering=False, debug=False)

    # Matmul operands live as float32r end-to-end: the host pre-rounds the
    # bits (round-to-nearest on the low 12 mantissa bits, matching walrus'
    # fp32_to_fp32r) so DMAs are plain copies and the PE runs at full rate.
    # All big tensors are laid out PARTITION-MAJOR on the host ([.., p, ..])
    # so every DMA moves one contiguous 8 KiB run per partition.
    reft = nc.dram_tensor(
        "reft", [n_st, 128, KB, ST_ROWS], f32r, kind="ExternalInput"
    ).ap()
    wrefT = nc.dram_tensor("wrefT", [128, KB, H], f32r, kind="ExternalInput").ap()
    qp = nc.dram_tensor("qp", [b_loc, H], f32, kind="ExternalInput").ap()
    vvec = nc.dram_tensor("vvec", [1, H], f32r, kind="ExternalInput").ap()
    # ref_proj leaves the device TRANSPOSED per supertile ([h', n]); the host
    # transposes it back while assembling. That keeps every DMA contiguous.
    refprojT = nc.dram_tensor(
        "refprojT", [n_st, 128, CB, ST_ROWS], f32, kind="ExternalOutput"
    ).ap()
    u = nc.dram_tensor("u", [b_loc, s_loc], f32, kind="ExternalOutput").ap()

    with tile.TileContext(nc) as tc, ExitStack() as ctx:
        const = ctx.enter_context(tc.tile_pool(name="const", bufs=1))
        qpp = ctx.enter_context(tc.tile_pool(name="qpp", bufs=3))
        inp = ctx.enter_context(tc.tile_pool(name="inp", bufs=6))
        outp = ctx.enter_context(tc.tile_pool(name="outp", bufs=6))
        tap = ctx.enter_context(tc.tile_pool(name="tap", bufs=6))
        urow = ctx.enter_context(tc.tile_pool(name="urow", bufs=4))
        psmm = ctx.enter_context(tc.tile_pool(name="psmm", bufs=6, space="PSUM"))
        psu = ctx.enter_context(tc.tile_pool(name="psu", bufs=2, space="PSUM"))

        # wrefT chunks [p(h), kb, h'] (stationary operands) and v as columns
        # [p(h'), cb] so the u-dot runs on the PE.
        wrefT_sb = const.tile([128, KB * H], f32r)
        nc.sync.dma_start(out=wrefT_sb, in_=wrefT.rearrange("p kb n -> p (kb n)"))
        v_col = const.tile([128, CB], f32r)
        nc.sync.dma_start(out=v_col, in_=vvec[0].rearrange("(c p) -> p c", p=128))

        for b in range(b_loc):
            # q_proj[b] as columns [p(h'), cb] -> per-partition tanh bias
            qp_sb = qpp.tile([128, CB], f32)
            nc.sync.dma_start(out=qp_sb, in_=qp[b].rearrange("(c p) -> p c", p=128))
            for st in range(st_per_batch):
                g = b * st_per_batch + st
                reft_sb = inp.tile([128, KB * ST_ROWS], f32r)
                nc.sync.dma_start(
                    out=reft_sb, in_=reft[g].rearrange("p kb n -> p (kb n)")
                )
                out_sb = outp.tile([128, CB * ST_ROWS], f32)
                u_ps = psu.tile([1, ST_ROWS], f32)
                for c in range(CB):
                    ps = psmm.tile([128, ST_ROWS], f32)
                    for kb in range(KB):
                        nc.tensor.matmul(
                            ps,
                            lhsT=wrefT_sb[
                                :, kb * H + c * 128 : kb * H + (c + 1) * 128
                            ],
                            rhs=reft_sb[:, kb * ST_ROWS : (kb + 1) * ST_ROWS],
                            start=(kb == 0),
                            stop=(kb == KB - 1),
                        )
                    nc.vector.tensor_copy(
                        out=out_sb[:, c * ST_ROWS : (c + 1) * ST_ROWS], in_=ps
                    )
                    th = tap.tile([128, ST_ROWS], f32r, tag="th")
                    nc.scalar.activation(
                        th,
                        ps,
                        mybir.ActivationFunctionType.Tanh,
                        bias=qp_sb[:, c : c + 1],
                    )
                    nc.tensor.matmul(
                        u_ps,
                        lhsT=v_col[:, c : c + 1],
                        rhs=th,
                        start=(c == 0),
                        stop=(c == CB - 1),
                    )
                nc.scalar.dma_start(
                    out=refprojT[g].rearrange("p c n -> p (c n)"), in_=out_sb
                )
                u_row = urow.tile([1, ST_ROWS], f32)
                nc.scalar.copy(out=u_row, in_=u_ps)
                nc.scalar.dma_start(
                    out=u[b : b + 1, st * ST_ROWS : (st + 1) * ST_ROWS], in_=u_row
                )

    nc.compile()
    return nc


def _get_program():
    key = (B_LOC, ST_PER_BATCH)
    if key not in _PROGRAM_CACHE:
        _PROGRAM_CACHE[key] = _build_program()
    return _PROGRAM_CACHE[key]


def _round_fp32r(x):
    """Round f32 to the fp32r-representable set (low 12 mantissa bits
    cleared, round-to-nearest) — bit-identical to walrus' fp32_to_fp32r."""
    b = x.view(np.uint32)
    rounded = (b + np.uint32(0x7FF) + ((b >> np.uint32(12)) & np.uint32(1))) & np.uint32(
        0xFFFFF000
    )
    return rounded.view(np.float32)


def kernel(q, ref, v, Wq, Wref):
    global LAST_RESULT
    import os
    from concourse.bass_utils import run_bass_kernel_spmd

    q = np.ascontiguousarray(np.asarray(q), dtype=np.float32)
    ref = np.ascontiguousarray(np.asarray(ref), dtype=np.float32)
    v = np.ascontiguousarray(np.asarray(v), dtype=np.float32)
    Wq = np.ascontiguousarray(np.asarray(Wq), dtype=np.float32)
    Wref = np.ascontiguousarray(np.asarray(Wref), dtype=np.float32)

    # Host-side prep (small): q projection, Wref transpose, ref supertile
    # transpose into partition-major layout [g, p, kb, n] so each partition's
    # DMA run on device is one contiguous 8 KiB.
    qp_full = np.ascontiguousarray(q @ Wq.T)                       # [B, H]
    wrefT = _round_fp32r(
        np.ascontiguousarray(Wref.T.reshape(KB, 128, H).transpose(1, 0, 2))
    )                                                              # [p, kb, h']
    reft_full = _round_fp32r(
        np.ascontiguousarray(
            ref.reshape(N_CORES * N_ST, ST_ROWS, KB, 128).transpose(0, 3, 2, 1)
        )
    )                                                              # [g, p, kb, n]

    vvec = _round_fp32r(v.reshape(1, H).copy())
    in_maps = [
        {
            "reft": reft_full[c * N_ST : (c + 1) * N_ST],
            "wrefT": wrefT,
            "qp": qp_full[c * B_LOC : (c + 1) * B_LOC],
            "vvec": vvec,
        }
        for c in range(N_CORES)
    ]

    nc = _get_program()
    res = run_bass_kernel_spmd(
        nc,
        in_maps,
        list(range(N_CORES)),
        tmpdir=os.environ.get("BASS_SPMD_TMPDIR"),
    )
    LAST_RESULT = res

    # refprojT per core: [g, p, c, n] with h' = c*128 + p, rows = g*512 + n.
    ref_proj = np.ascontiguousarray(
        np.stack([res.results[c]["refprojT"] for c in range(N_CORES)]).transpose(
            0, 1, 4, 3, 2
        )
    ).reshape(B, S, H)
    u = np.concatenate([res.results[c]["u"] for c in range(N_CORES)], axis=0)
    return u, ref_proj


# revision 28
# speedup vs baseline: 1.0571x; 1.0571x over previous
"""Trainium2 Bass kernel for nn_Attention_3633542333119 (additive attention).

reference:
    q_proj   = q @ Wq.T                          # [B, H]
    ref_proj = (ref @ Wref.T).reshape(B, S, H)   # [B, S, H]
    u        = einsum("bsh,h->bs", tanh(q_proj[:, None, :] + ref_proj), v)
    return (u, ref_proj)

B=128, S=2048, H=512. Data-parallel over batch across 8 NeuronCores
(16 batches / 32768 ref rows per core); Wref/v replicated; q_proj is
precomputed on host (tiny: 128x512 @ 512x512) and sharded with the batch.

Device-side plan (per core), all f32, matmuls in fp32r (full-rate):
  - host pre-transposes ref into supertiles reft[g] = ref[g*512:(g+1)*512].T
    (shape [512h, 512n]) so the PE's stationary operand (lhsT, [K=h, M=n])
    loads with plain DMA - no on-device transpose needed.
  - per supertile: psum[mb] = sum_kb reft_blk(kb,mb).T @ wrefT_blk(kb)
    (4 accumulating matmuls, N=512) -> ref_proj tile [128n, 512h'].
  - epilogue: ACT copies psum->SBUF (ref_proj out), DVE adds broadcast
    q_proj row, ACT tanh, DVE fused (tanh*v)+reduce -> u column.
  - u columns staged per batch [128, 16], PE-transposed, DMA'd out.
"""

import numpy as np
from contextlib import ExitStack

B = 128
S = 2048
H = 512
KB = H // 128                   # 128-row chunks of the hidden dim
N_CORES = 8
B_LOC = B // N_CORES            # batches per core
ST_ROWS = 512                   # rows (n) per supertile
ST_PER_BATCH = S // ST_ROWS     # supertiles per batch
N_ST = B_LOC * ST_PER_BATCH     # supertiles per core
ROWS = B_LOC * S                # ref rows per core

_PROGRAM_CACHE = {}
LAST_RESULT = None              # BassKernelResults of the last kernel() call


def _build_program(b_loc=B_LOC, st_per_batch=ST_PER_BATCH):
    import concourse.bass as bass
    import concourse.tile as tile
    import concourse.mybir as mybir
    from concourse import bacc

    f32 = mybir.dt.float32
    f32r = mybir.dt.float32r
    KB = H // 128               # contraction (h) chunks (4)
    CB = H // 128               # output (h') chunks per supertile (4)
    n_st = b_loc * st_per_batch
    s_loc = st_per_batch * ST_ROWS

    nc = bacc.Bacc("TRN2", target_bir_lowering=False, debug=False)

    # Matmul operands live as float32r end-to-end: the host pre-rounds the
    # bits (round-to-nearest on the low 12 mantissa bits, matching walrus'
    # fp32_to_fp32r) so DMAs are plain copies and the PE runs at full rate.
    # All big tensors are laid out PARTITION-MAJOR on the host ([.., p, ..])
    # so every DMA moves one contiguous 8 KiB run per partition.
    reft = nc.dram_tensor(
        "reft", [n_st, 128, KB, ST_ROWS], f32r, kind="ExternalInput"
    ).ap()
    wrefT = nc.dram_tensor("wrefT", [128, KB, H], f32r, kind="ExternalInput").ap()
    qp = nc.dram_tensor("qp", [b_loc, H], f32, kind="ExternalInput").ap()
    vvec = nc.dram_tensor("vvec", [1, H], f32r, kind="ExternalInput").ap()
    # ref_proj leaves the device TRANSPOSED per supertile ([h', n]); the host
    # transposes it back while assembling. That keeps every DMA contiguous.
    refprojT = nc.dram_tensor(
        "refprojT", [n_st, 128, CB, ST_ROWS], f32, kind="ExternalOutput"
    ).ap()
    u = nc.dram_tensor("u", [b_loc, s_loc], f32, kind="ExternalOutput").ap()

    with tile.TileContext(nc) as tc, ExitStack() as ctx:
        const = ctx.enter_context(tc.tile_pool(name="const", bufs=1))
        qpp = ctx.enter_context(tc.tile_pool(name="qpp", bufs=3))
        inp = ctx.enter_context(tc.tile_pool(name="inp", bufs=6))
        outp = ctx.enter_context(tc.tile_pool(name="outp", bufs=6))
        tap = ctx.enter_context(tc.tile_pool(name="tap", bufs=6))
        urow = ctx.enter_context(tc.tile_pool(name="urow", bufs=4))
        psmm = ctx.enter_context(tc.tile_pool(name="psmm", bufs=6, space="PSUM"))
        psu = ctx.enter_context(tc.tile_pool(name="psu", bufs=2, space="PSUM"))

        # wrefT chunks [p(h), kb, h'] (stationary operands) and v as columns
        # [p(h'), cb] so the u-dot runs on the PE.
        wrefT_sb = const.tile([128, KB * H], f32r)
        nc.sync.dma_start(out=wrefT_sb, in_=wrefT.rearrange("p kb n -> p (kb n)"))
        v_col = const.tile([128, CB], f32r)
        nc.sync.dma_start(out=v_col, in_=vvec[0].rearrange("(c p) -> p c", p=128))
        # all q_proj rows as columns [p(h'), (b, c)] -> per-partition tanh bias;
        # one small gather up front keeps tiny descriptors out of the stream.
        qp_all = const.tile([128, b_loc * CB], f32)
        nc.sync.dma_start(
            out=qp_all, in_=qp.rearrange("b (c p) -> p (b c)", p=128)
        )

        for b in range(b_loc):
            u_sb = urow.tile([1, s_loc], f32)
            for st in range(st_per_batch):
                g = b * st_per_batch + st
                reft_sb = inp.tile([128, KB * ST_ROWS], f32r)
                nc.sync.dma_start(
                    out=reft_sb, in_=reft[g].rearrange("p kb n -> p (kb n)")
                )
                out_sb = outp.tile([128, CB * ST_ROWS], f32)
                u_ps = psu.tile([1, ST_ROWS], f32)
                for c in range(CB):
                    ps = psmm.tile([128, ST_ROWS], f32)
                    for kb in range(KB):
                        nc.tensor.matmul(
                            ps,
                            lhsT=wrefT_sb[
                                :, kb * H + c * 128 : kb * H + (c + 1) * 128
                            ],
                            rhs=reft_sb[:, kb * ST_ROWS : (kb + 1) * ST_ROWS],
                            start=(kb == 0),
                            stop=(kb == KB - 1),
                        )
                    nc.vector.tensor_copy(
                        out=out_sb[:, c * ST_ROWS : (c + 1) * ST_ROWS], in_=ps
                    )
                    th = tap.tile([128, ST_ROWS], f32r, tag="th")
                    nc.scalar.activation(
                        th,
                        ps,
                        mybir.ActivationFunctionType.Tanh,
                        bias=qp_all[:, b * CB + c : b * CB + c + 1],
                    )
                    nc.tensor.matmul(
                        u_ps,
                        lhsT=v_col[:, c : c + 1],
                        rhs=th,
                        start=(c == 0),
                        stop=(c == CB - 1),
                    )
                nc.scalar.dma_start(
                    out=refprojT[g].rearrange("p c n -> p (c n)"), in_=out_sb
                )
                nc.scalar.copy(
                    out=u_sb[:, st * ST_ROWS : (st + 1) * ST_ROWS], in_=u_ps
                )
            nc.scalar.dma_start(out=u[b : b + 1, :], in_=u_sb)

    nc.compile()
    return nc


def _get_program():
    key = (B_LOC, ST_PER_BATCH)
    if key not in _PROGRAM_CACHE:
        _PROGRAM_CACHE[key] = _build_program()
    return _PROGRAM_CACHE[key]


def _round_fp32r(x):
    """Round f32 to the fp32r-representable set (low 12 mantissa bits
    cleared, round-to-nearest) — bit-identical to walrus' fp32_to_fp32r."""
    b = x.view(np.uint32)
    rounded = (b + np.uint32(0x7FF) + ((b >> np.uint32(12)) & np.uint32(1))) & np.uint32(
        0xFFFFF000
    )
    return rounded.view(np.float32)


def kernel(q, ref, v, Wq, Wref):
    global LAST_RESULT
    import os
    from concourse.bass_utils import run_bass_kernel_spmd

    q = np.ascontiguousarray(np.asarray(q), dtype=np.float32)
    ref = np.ascontiguousarray(np.asarray(ref), dtype=np.float32)
    v = np.ascontiguousarray(np.asarray(v), dtype=np.float32)
    Wq = np.ascontiguousarray(np.asarray(Wq), dtype=np.float32)
    Wref = np.ascontiguousarray(np.asarray(Wref), dtype=np.float32)

    # Host-side prep (small): q projection, Wref transpose, ref supertile
    # transpose into partition-major layout [g, p, kb, n] so each partition's
    # DMA run on device is one contiguous 8 KiB.
    qp_full = np.ascontiguousarray(q @ Wq.T)                       # [B, H]
    wrefT = _round_fp32r(
        np.ascontiguousarray(Wref.T.reshape(KB, 128, H).transpose(1, 0, 2))
    )                                                              # [p, kb, h']
    reft_full = _round_fp32r(
        np.ascontiguousarray(
            ref.reshape(N_CORES * N_ST, ST_ROWS, KB, 128).transpose(0, 3, 2, 1)
        )
    )                                                              # [g, p, kb, n]

    vvec = _round_fp32r(v.reshape(1, H).copy())
    in_maps = [
        {
            "reft": reft_full[c * N_ST : (c + 1) * N_ST],
            "wrefT": wrefT,
            "qp": qp_full[c * B_LOC : (c + 1) * B_LOC],
            "vvec": vvec,
        }
        for c in range(N_CORES)
    ]

    nc = _get_program()
    res = run_bass_kernel_spmd(
        nc,
        in_maps,
        list(range(N_CORES)),
        tmpdir=os.environ.get("BASS_SPMD_TMPDIR"),
    )
    LAST_RESULT = res

    # refprojT per core: [g, p, c, n] with h' = c*128 + p, rows = g*512 + n.
    ref_proj = np.ascontiguousarray(
        np.stack([res.results[c]["refprojT"] for c in range(N_CORES)]).transpose(
            0, 1, 4, 3, 2
        )
    ).reshape(B, S, H)
    u = np.concatenate([res.results[c]["u"] for c in range(N_CORES)], axis=0)
    return u, ref_proj


# revision 30
# speedup vs baseline: 1.1152x; 1.0549x over previous
"""Trainium2 Bass kernel for nn_Attention_3633542333119 (additive attention).

reference:
    q_proj   = q @ Wq.T                          # [B, H]
    ref_proj = (ref @ Wref.T).reshape(B, S, H)   # [B, S, H]
    u        = einsum("bsh,h->bs", tanh(q_proj[:, None, :] + ref_proj), v)
    return (u, ref_proj)

B=128, S=2048, H=512. Data-parallel over batch across 8 NeuronCores
(16 batches / 32768 ref rows per core); Wref/v replicated; q_proj is
precomputed on host (tiny: 128x512 @ 512x512) and sharded with the batch.

Device-side plan (per core), all f32, matmuls in fp32r (full-rate):
  - host pre-transposes ref into supertiles reft[g] = ref[g*512:(g+1)*512].T
    (shape [512h, 512n]) so the PE's stationary operand (lhsT, [K=h, M=n])
    loads with plain DMA - no on-device transpose needed.
  - per supertile: psum[mb] = sum_kb reft_blk(kb,mb).T @ wrefT_blk(kb)
    (4 accumulating matmuls, N=512) -> ref_proj tile [128n, 512h'].
  - epilogue: ACT copies psum->SBUF (ref_proj out), DVE adds broadcast
    q_proj row, ACT tanh, DVE fused (tanh*v)+reduce -> u column.
  - u columns staged per batch [128, 16], PE-transposed, DMA'd out.
"""

import numpy as np
from contextlib import ExitStack

B = 128
S = 2048
H = 512
KB = H // 128                   # 128-row chunks of the hidden dim
N_CORES = 8
B_LOC = B // N_CORES            # batches per core
ST_ROWS = 512                   # rows (n) per supertile
ST_PER_BATCH = S // ST_ROWS     # supertiles per batch
N_ST = B_LOC * ST_PER_BATCH     # supertiles per core
ROWS = B_LOC * S                # ref rows per core

_PROGRAM_CACHE = {}
LAST_RESULT = None              # BassKernelResults of the last kernel() call


def _build_program(b_loc=B_LOC, st_per_batch=ST_PER_BATCH):
    import concourse.bass as bass
    import concourse.tile as tile
    import concourse.mybir as mybir
    from concourse import bacc

    f32 = mybir.dt.float32
    f32r = mybir.dt.float32r
    KB = H // 128               # contraction (h) chunks (4)
    CB = H // 128               # output (h') chunks per supertile (4)
    n_st = b_loc * st_per_batch
    s_loc = st_per_batch * ST_ROWS

    nc = bacc.Bacc("TRN2", target_bir_lowering=False, debug=False)

    # Matmul operands live as float32r end-to-end: the host pre-rounds the
    # bits (round-to-nearest on the low 12 mantissa bits, matching walrus'
    # fp32_to_fp32r) so DMAs are plain copies and the PE runs at full rate.
    # All big tensors are laid out PARTITION-MAJOR on the host ([.., p, ..])
    # so every DMA moves one contiguous 8 KiB run per partition.
    reft = nc.dram_tensor(
        "reft", [n_st, 128, KB, ST_ROWS], f32r, kind="ExternalInput"
    ).ap()
    wrefT = nc.dram_tensor("wrefT", [128, KB, H], f32r, kind="ExternalInput").ap()
    qp = nc.dram_tensor("qp", [b_loc, H], f32, kind="ExternalInput").ap()
    vvec = nc.dram_tensor("vvec", [1, H], f32r, kind="ExternalInput").ap()
    # ref_proj leaves the device TRANSPOSED per supertile ([h', n]); the host
    # transposes it back while assembling. That keeps every DMA contiguous.
    refprojT = nc.dram_tensor(
        "refprojT", [n_st, 128, CB, ST_ROWS], f32, kind="ExternalOutput"
    ).ap()
    u = nc.dram_tensor("u", [b_loc, s_loc], f32, kind="ExternalOutput").ap()

    with tile.TileContext(nc) as tc, ExitStack() as ctx:
        const = ctx.enter_context(tc.tile_pool(name="const", bufs=1))
        qpp = ctx.enter_context(tc.tile_pool(name="qpp", bufs=3))
        inp = ctx.enter_context(tc.tile_pool(name="inp", bufs=6))
        outp = ctx.enter_context(tc.tile_pool(name="outp", bufs=6))
        tap = ctx.enter_context(tc.tile_pool(name="tap", bufs=6))
        urow = ctx.enter_context(tc.tile_pool(name="urow", bufs=4))
        psmm = ctx.enter_context(tc.tile_pool(name="psmm", bufs=7, space="PSUM"))
        psu = ctx.enter_context(tc.tile_pool(name="psu", bufs=1, space="PSUM"))

        # wrefT chunks [p(h), kb, h'] (stationary operands) and v as columns
        # [p(h'), cb] so the u-dot runs on the PE.
        wrefT_sb = const.tile([128, KB * H], f32r)
        nc.sync.dma_start(out=wrefT_sb, in_=wrefT.rearrange("p kb n -> p (kb n)"))
        v_col = const.tile([128, CB], f32r)
        nc.sync.dma_start(out=v_col, in_=vvec[0].rearrange("(c p) -> p c", p=128))
        # all q_proj rows as columns [p(h'), (b, c)] -> per-partition tanh bias;
        # one small gather up front keeps tiny descriptors out of the stream.
        qp_all = const.tile([128, b_loc * CB], f32)
        nc.sync.dma_start(
            out=qp_all, in_=qp.rearrange("b (c p) -> p (b c)", p=128)
        )

        for b in range(b_loc):
            u_sb = urow.tile([1, s_loc], f32)
            for st in range(st_per_batch):
                g = b * st_per_batch + st
                reft_sb = inp.tile([128, KB * ST_ROWS], f32r)
                nc.sync.dma_start(
                    out=reft_sb, in_=reft[g].rearrange("p kb n -> p (kb n)")
                )
                out_sb = outp.tile([128, CB * ST_ROWS], f32)
                u_ps = psu.tile([1, ST_ROWS], f32)
                for c in range(CB):
                    ps = psmm.tile([128, ST_ROWS], f32)
                    for kb in range(KB):
                        nc.tensor.matmul(
                            ps,
                            lhsT=wrefT_sb[
                                :, kb * H + c * 128 : kb * H + (c + 1) * 128
                            ],
                            rhs=reft_sb[:, kb * ST_ROWS : (kb + 1) * ST_ROWS],
                            start=(kb == 0),
                            stop=(kb == KB - 1),
                        )
                    nc.vector.tensor_copy(
                        out=out_sb[:, c * ST_ROWS : (c + 1) * ST_ROWS], in_=ps
                    )
                    th = tap.tile([128, ST_ROWS], f32r, tag="th")
                    nc.scalar.activation(
                        th,
                        ps,
                        mybir.ActivationFunctionType.Tanh,
                        bias=qp_all[:, b * CB + c : b * CB + c + 1],
                    )
                    nc.tensor.matmul(
                        u_ps,
                        lhsT=v_col[:, c : c + 1],
                        rhs=th,
                        start=(c == 0),
                        stop=(c == CB - 1),
                    )
                    if c % 2 == 1:  # store per chunk-pair: drains earlier
                        h0 = (c - 1) * ST_ROWS
                        h1 = (c + 1) * ST_ROWS
                        nc.scalar.dma_start(
                            out=refprojT[g].rearrange("p c n -> p (c n)")[:, h0:h1],
                            in_=out_sb[:, h0:h1],
                        )
                nc.scalar.copy(
                    out=u_sb[:, st * ST_ROWS : (st + 1) * ST_ROWS], in_=u_ps
                )
            nc.scalar.dma_start(out=u[b : b + 1, :], in_=u_sb)

    nc.compile()
    return nc


def _get_program():
    key = (B_LOC, ST_PER_BATCH)
    if key not in _PROGRAM_CACHE:
        _PROGRAM_CACHE[key] = _build_program()
    return _PROGRAM_CACHE[key]


def _round_fp32r(x):
    """Round f32 to the fp32r-representable set (low 12 mantissa bits
    cleared, round-to-nearest) — bit-identical to walrus' fp32_to_fp32r."""
    b = x.view(np.uint32)
    rounded = (b + np.uint32(0x7FF) + ((b >> np.uint32(12)) & np.uint32(1))) & np.uint32(
        0xFFFFF000
    )
    return rounded.view(np.float32)


def kernel(q, ref, v, Wq, Wref):
    global LAST_RESULT
    import os
    from concourse.bass_utils import run_bass_kernel_spmd

    q = np.ascontiguousarray(np.asarray(q), dtype=np.float32)
    ref = np.ascontiguousarray(np.asarray(ref), dtype=np.float32)
    v = np.ascontiguousarray(np.asarray(v), dtype=np.float32)
    Wq = np.ascontiguousarray(np.asarray(Wq), dtype=np.float32)
    Wref = np.ascontiguousarray(np.asarray(Wref), dtype=np.float32)

    # Host-side prep (small): q projection, Wref transpose, ref supertile
    # transpose into partition-major layout [g, p, kb, n] so each partition's
    # DMA run on device is one contiguous 8 KiB.
    qp_full = np.ascontiguousarray(q @ Wq.T)                       # [B, H]
    wrefT = _round_fp32r(
        np.ascontiguousarray(Wref.T.reshape(KB, 128, H).transpose(1, 0, 2))
    )                                                              # [p, kb, h']
    reft_full = _round_fp32r(
        np.ascontiguousarray(
            ref.reshape(N_CORES * N_ST, ST_ROWS, KB, 128).transpose(0, 3, 2, 1)
        )
    )                                                              # [g, p, kb, n]

    vvec = _round_fp32r(v.reshape(1, H).copy())
    in_maps = [
        {
            "reft": reft_full[c * N_ST : (c + 1) * N_ST],
            "wrefT": wrefT,
            "qp": qp_full[c * B_LOC : (c + 1) * B_LOC],
            "vvec": vvec,
        }
        for c in range(N_CORES)
    ]

    nc = _get_program()
    res = run_bass_kernel_spmd(
        nc,
        in_maps,
        list(range(N_CORES)),
        tmpdir=os.environ.get("BASS_SPMD_TMPDIR"),
    )
    LAST_RESULT = res

    # refprojT per core: [g, p, c, n] with h' = c*128 + p, rows = g*512 + n.
    ref_proj = np.ascontiguousarray(
        np.stack([res.results[c]["refprojT"] for c in range(N_CORES)]).transpose(
            0, 1, 4, 3, 2
        )
    ).reshape(B, S, H)
    u = np.concatenate([res.results[c]["u"] for c in range(N_CORES)], axis=0)
    return u, ref_proj


# revision 35
# speedup vs baseline: 1.2735x; 1.1420x over previous
"""Trainium2 Bass kernel for nn_Attention_3633542333119 (additive attention).

reference:
    q_proj   = q @ Wq.T                          # [B, H]
    ref_proj = (ref @ Wref.T).reshape(B, S, H)   # [B, S, H]
    u        = einsum("bsh,h->bs", tanh(q_proj[:, None, :] + ref_proj), v)
    return (u, ref_proj)

B=128, S=2048, H=512. Data-parallel over batch across 8 NeuronCores
(16 batches / 32768 ref rows per core); Wref/v replicated; q_proj is
precomputed on host (tiny: 128x512 @ 512x512) and sharded with the batch.

Device-side plan (per core), all f32, matmuls in fp32r (full-rate):
  - host pre-transposes ref into supertiles reft[g] = ref[g*512:(g+1)*512].T
    (shape [512h, 512n]) so the PE's stationary operand (lhsT, [K=h, M=n])
    loads with plain DMA - no on-device transpose needed.
  - per supertile: psum[mb] = sum_kb reft_blk(kb,mb).T @ wrefT_blk(kb)
    (4 accumulating matmuls, N=512) -> ref_proj tile [128n, 512h'].
  - epilogue: ACT copies psum->SBUF (ref_proj out), DVE adds broadcast
    q_proj row, ACT tanh, DVE fused (tanh*v)+reduce -> u column.
  - u columns staged per batch [128, 16], PE-transposed, DMA'd out.
"""

import numpy as np
from contextlib import ExitStack

B = 128
S = 2048
H = 512
KB = H // 128                   # 128-row chunks of the hidden dim
N_CORES = 8
B_LOC = B // N_CORES            # batches per core
ST_ROWS = 512                   # rows (n) per supertile
ST_PER_BATCH = S // ST_ROWS     # supertiles per batch
N_ST = B_LOC * ST_PER_BATCH     # supertiles per core
ROWS = B_LOC * S                # ref rows per core

_PROGRAM_CACHE = {}
LAST_RESULT = None              # BassKernelResults of the last kernel() call


# Input dtype for the big matmul operands: "f32r" (fp32 @ 11-bit mantissa,
# full-rate PE, full-size DMA) or "f16" (10-bit mantissa, half the ref DMA
# traffic + fast weight loads). Both keep f32 outputs.
IN_DTYPE = "f16"


def _build_program(b_loc=B_LOC, st_per_batch=ST_PER_BATCH, in_dtype=None):
    import concourse.bass as bass
    import concourse.tile as tile
    import concourse.mybir as mybir
    from concourse import bacc

    if in_dtype is None:
        in_dtype = IN_DTYPE
    f32 = mybir.dt.float32
    f32r = mybir.dt.float32r
    fin = mybir.dt.float16 if in_dtype == "f16" else f32r
    KB = H // 128               # contraction (h) chunks (4)
    CB = H // 128               # output (h') chunks per supertile (4)
    n_st = b_loc * st_per_batch
    s_loc = st_per_batch * ST_ROWS

    nc = bacc.Bacc("TRN2", target_bir_lowering=False, debug=False)

    # Matmul operands live as float32r end-to-end: the host pre-rounds the
    # bits (round-to-nearest on the low 12 mantissa bits, matching walrus'
    # fp32_to_fp32r) so DMAs are plain copies and the PE runs at full rate.
    # All big tensors are laid out PARTITION-MAJOR on the host ([.., p, ..])
    # so every DMA moves one contiguous 8 KiB run per partition.
    reft = nc.dram_tensor(
        "reft", [n_st, 128, KB, ST_ROWS], fin, kind="ExternalInput"
    ).ap()
    wrefT = nc.dram_tensor("wrefT", [128, KB, H], fin, kind="ExternalInput").ap()
    qp = nc.dram_tensor("qp", [b_loc, H], f32, kind="ExternalInput").ap()
    vvec = nc.dram_tensor("vvec", [1, H], f32r, kind="ExternalInput").ap()
    # ref_proj leaves the device TRANSPOSED per supertile ([h', n]); the host
    # transposes it back while assembling. That keeps every DMA contiguous.
    refprojT = nc.dram_tensor(
        "refprojT", [n_st, 128, CB, ST_ROWS], f32, kind="ExternalOutput"
    ).ap()
    u = nc.dram_tensor("u", [b_loc, s_loc], f32, kind="ExternalOutput").ap()

    with tile.TileContext(nc) as tc, ExitStack() as ctx:
        const = ctx.enter_context(tc.tile_pool(name="const", bufs=1))
        qpp = ctx.enter_context(tc.tile_pool(name="qpp", bufs=3))
        inp = ctx.enter_context(tc.tile_pool(name="inp", bufs=6))
        outp = ctx.enter_context(tc.tile_pool(name="outp", bufs=6))
        tap = ctx.enter_context(tc.tile_pool(name="tap", bufs=6))
        urow = ctx.enter_context(tc.tile_pool(name="urow", bufs=4))
        psmm = ctx.enter_context(tc.tile_pool(name="psmm", bufs=7, space="PSUM"))
        psu = ctx.enter_context(tc.tile_pool(name="psu", bufs=1, space="PSUM"))

        # wrefT chunks [p(h), kb, h'] (stationary operands) and v as columns
        # [p(h'), cb] so the u-dot runs on the PE.
        wrefT_sb = const.tile([128, KB * H], fin)
        nc.sync.dma_start(out=wrefT_sb, in_=wrefT.rearrange("p kb n -> p (kb n)"))
        v_col = const.tile([128, CB], f32r)
        nc.sync.dma_start(out=v_col, in_=vvec[0].rearrange("(c p) -> p c", p=128))
        # all q_proj rows as columns [p(h'), (b, c)] -> per-partition tanh bias;
        # one small gather up front keeps tiny descriptors out of the stream.
        qp_all = const.tile([128, b_loc * CB], f32)
        nc.sync.dma_start(
            out=qp_all, in_=qp.rearrange("b (c p) -> p (b c)", p=128)
        )

        for b in range(b_loc):
            u_sb = urow.tile([1, s_loc], f32)
            for st in range(st_per_batch):
                g = b * st_per_batch + st
                reft_sb = inp.tile([128, KB * ST_ROWS], fin)
                nc.sync.dma_start(
                    out=reft_sb, in_=reft[g].rearrange("p kb n -> p (kb n)")
                )
                out_sb = outp.tile([128, CB * ST_ROWS], f32)
                u_ps = psu.tile([1, ST_ROWS], f32)
                for c in range(CB):
                    ps = psmm.tile([128, ST_ROWS], f32)
                    for kb in range(KB):
                        nc.tensor.matmul(
                            ps,
                            lhsT=wrefT_sb[
                                :, kb * H + c * 128 : kb * H + (c + 1) * 128
                            ],
                            rhs=reft_sb[:, kb * ST_ROWS : (kb + 1) * ST_ROWS],
                            start=(kb == 0),
                            stop=(kb == KB - 1),
                        )
                    nc.vector.tensor_copy(
                        out=out_sb[:, c * ST_ROWS : (c + 1) * ST_ROWS], in_=ps
                    )
                    th = tap.tile([128, ST_ROWS], f32r, tag="th")
                    nc.scalar.activation(
                        th,
                        ps,
                        mybir.ActivationFunctionType.Tanh,
                        bias=qp_all[:, b * CB + c : b * CB + c + 1],
                    )
                    nc.tensor.matmul(
                        u_ps,
                        lhsT=v_col[:, c : c + 1],
                        rhs=th,
                        start=(c == 0),
                        stop=(c == CB - 1),
                    )
                    if c % 2 == 1:  # store per chunk-pair: drains earlier
                        h0 = (c - 1) * ST_ROWS
                        h1 = (c + 1) * ST_ROWS
                        nc.scalar.dma_start(
                            out=refprojT[g].rearrange("p c n -> p (c n)")[:, h0:h1],
                            in_=out_sb[:, h0:h1],
                        )
                nc.scalar.copy(
                    out=u_sb[:, st * ST_ROWS : (st + 1) * ST_ROWS], in_=u_ps
                )
            nc.scalar.dma_start(out=u[b : b + 1, :], in_=u_sb)

    nc.compile()
    return nc


def _get_program():
    key = (B_LOC, ST_PER_BATCH)
    if key not in _PROGRAM_CACHE:
        _PROGRAM_CACHE[key] = _build_program()
    return _PROGRAM_CACHE[key]


def _round_fp32r(x):
    """Round f32 to the fp32r-representable set (low 12 mantissa bits
    cleared, round-to-nearest) — bit-identical to walrus' fp32_to_fp32r."""
    b = x.view(np.uint32)
    rounded = (b + np.uint32(0x7FF) + ((b >> np.uint32(12)) & np.uint32(1))) & np.uint32(
        0xFFFFF000
    )
    return rounded.view(np.float32)


def kernel(q, ref, v, Wq, Wref):
    global LAST_RESULT
    import os
    from concourse.bass_utils import run_bass_kernel_spmd

    q = np.ascontiguousarray(np.asarray(q), dtype=np.float32)
    ref = np.ascontiguousarray(np.asarray(ref), dtype=np.float32)
    v = np.ascontiguousarray(np.asarray(v), dtype=np.float32)
    Wq = np.ascontiguousarray(np.asarray(Wq), dtype=np.float32)
    Wref = np.ascontiguousarray(np.asarray(Wref), dtype=np.float32)

    # Host-side prep (small): q projection, Wref transpose, ref supertile
    # transpose into partition-major layout [g, p, kb, n] so each partition's
    # DMA run on device is one contiguous 8 KiB.
    qp_full = np.ascontiguousarray(q @ Wq.T)                       # [B, H]
    wrefT = np.ascontiguousarray(
        Wref.T.reshape(KB, 128, H).transpose(1, 0, 2)
    )                                                              # [p, kb, h']
    reft_full = np.ascontiguousarray(
        ref.reshape(N_CORES * N_ST, ST_ROWS, KB, 128).transpose(0, 3, 2, 1)
    )                                                              # [g, p, kb, n]
    if IN_DTYPE == "f16":
        wrefT = wrefT.astype(np.float16)
        reft_full = reft_full.astype(np.float16)
    else:
        wrefT = _round_fp32r(wrefT)
        reft_full = _round_fp32r(reft_full)

    vvec = _round_fp32r(v.reshape(1, H).copy())
    in_maps = [
        {
            "reft": reft_full[c * N_ST : (c + 1) * N_ST],
            "wrefT": wrefT,
            "qp": qp_full[c * B_LOC : (c + 1) * B_LOC],
            "vvec": vvec,
        }
        for c in range(N_CORES)
    ]

    nc = _get_program()
    res = run_bass_kernel_spmd(
        nc,
        in_maps,
        list(range(N_CORES)),
        tmpdir=os.environ.get("BASS_SPMD_TMPDIR"),
    )
    LAST_RESULT = res

    # refprojT per core: [g, p, c, n] with h' = c*128 + p, rows = g*512 + n.
    ref_proj = np.ascontiguousarray(
        np.stack([res.results[c]["refprojT"] for c in range(N_CORES)]).transpose(
            0, 1, 4, 3, 2
        )
    ).reshape(B, S, H)
    u = np.concatenate([res.results[c]["u"] for c in range(N_CORES)], axis=0)
    return u, ref_proj


# revision 41
# speedup vs baseline: 1.3228x; 1.0387x over previous
"""Trainium2 Bass kernel for nn_Attention_3633542333119 (additive attention).

reference:
    q_proj   = q @ Wq.T                          # [B, H]
    ref_proj = (ref @ Wref.T).reshape(B, S, H)   # [B, S, H]
    u        = einsum("bsh,h->bs", tanh(q_proj[:, None, :] + ref_proj), v)
    return (u, ref_proj)

B=128, S=2048, H=512. Data-parallel over batch across 8 NeuronCores
(16 batches / 32768 ref rows per core); Wref/v replicated; q_proj is
precomputed on host (tiny: 128x512 @ 512x512) and sharded with the batch.

Device-side plan (per core), all f32, matmuls in fp32r (full-rate):
  - host pre-transposes ref into supertiles reft[g] = ref[g*512:(g+1)*512].T
    (shape [512h, 512n]) so the PE's stationary operand (lhsT, [K=h, M=n])
    loads with plain DMA - no on-device transpose needed.
  - per supertile: psum[mb] = sum_kb reft_blk(kb,mb).T @ wrefT_blk(kb)
    (4 accumulating matmuls, N=512) -> ref_proj tile [128n, 512h'].
  - epilogue: ACT copies psum->SBUF (ref_proj out), DVE adds broadcast
    q_proj row, ACT tanh, DVE fused (tanh*v)+reduce -> u column.
  - u columns staged per batch [128, 16], PE-transposed, DMA'd out.
"""

import numpy as np
from contextlib import ExitStack

B = 128
S = 2048
H = 512
KB = H // 128                   # 128-row chunks of the hidden dim
N_CORES = 8
B_LOC = B // N_CORES            # batches per core
ST_ROWS = 512                   # rows (n) per supertile
ST_PER_BATCH = S // ST_ROWS     # supertiles per batch
N_ST = B_LOC * ST_PER_BATCH     # supertiles per core
ROWS = B_LOC * S                # ref rows per core

_PROGRAM_CACHE = {}
LAST_RESULT = None              # BassKernelResults of the last kernel() call


# Input dtype for the big matmul operands: "f32r" (fp32 @ 11-bit mantissa,
# full-rate PE, full-size DMA) or "f16" (10-bit mantissa, half the ref DMA
# traffic + fast weight loads). Both keep f32 outputs.
IN_DTYPE = "f16"


def _build_program(b_loc=B_LOC, st_per_batch=ST_PER_BATCH, in_dtype=None):
    import concourse.bass as bass
    import concourse.tile as tile
    import concourse.mybir as mybir
    from concourse import bacc

    if in_dtype is None:
        in_dtype = IN_DTYPE
    f32 = mybir.dt.float32
    f32r = mybir.dt.float32r
    fin = mybir.dt.float16 if in_dtype == "f16" else f32r
    KB = H // 128               # contraction (h) chunks (4)
    CB = H // 128               # output (h') chunks per supertile (4)
    n_st = b_loc * st_per_batch
    s_loc = st_per_batch * ST_ROWS

    nc = bacc.Bacc("TRN2", target_bir_lowering=False, debug=False)

    # Matmul operands live as float32r end-to-end: the host pre-rounds the
    # bits (round-to-nearest on the low 12 mantissa bits, matching walrus'
    # fp32_to_fp32r) so DMAs are plain copies and the PE runs at full rate.
    # All big tensors are laid out PARTITION-MAJOR on the host ([.., p, ..])
    # so every DMA moves one contiguous 8 KiB run per partition.
    assert n_st % 2 == 0
    reft = nc.dram_tensor(
        "reft", [n_st // 2, 128, 2, KB, ST_ROWS], fin, kind="ExternalInput"
    ).ap()
    wrefT = nc.dram_tensor("wrefT", [128, KB, H], fin, kind="ExternalInput").ap()
    qp = nc.dram_tensor("qp", [b_loc, H], f32, kind="ExternalInput").ap()
    vvec = nc.dram_tensor("vvec", [1, H], f32r, kind="ExternalInput").ap()
    # ref_proj leaves the device TRANSPOSED per supertile ([h', n]); the host
    # transposes it back while assembling. That keeps every DMA contiguous.
    refprojT = nc.dram_tensor(
        "refprojT", [n_st, 128, CB, ST_ROWS], f32, kind="ExternalOutput"
    ).ap()
    u = nc.dram_tensor("u", [b_loc, s_loc], f32, kind="ExternalOutput").ap()

    with tile.TileContext(nc) as tc, ExitStack() as ctx:
        const = ctx.enter_context(tc.tile_pool(name="const", bufs=1))
        qpp = ctx.enter_context(tc.tile_pool(name="qpp", bufs=3))
        inp = ctx.enter_context(tc.tile_pool(name="inp", bufs=6))
        outp = ctx.enter_context(tc.tile_pool(name="outp", bufs=6))
        tap = ctx.enter_context(tc.tile_pool(name="tap", bufs=6))
        urow = ctx.enter_context(tc.tile_pool(name="urow", bufs=4))
        psmm = ctx.enter_context(tc.tile_pool(name="psmm", bufs=7, space="PSUM"))
        psu = ctx.enter_context(tc.tile_pool(name="psu", bufs=1, space="PSUM"))

        # wrefT chunks [p(h), kb, h'] (stationary operands) and v as columns
        # [p(h'), cb] so the u-dot runs on the PE.
        wrefT_sb = const.tile([128, KB * H], fin)
        nc.sync.dma_start(out=wrefT_sb, in_=wrefT.rearrange("p kb n -> p (kb n)"))
        v_col = const.tile([128, CB], f32r)
        nc.sync.dma_start(out=v_col, in_=vvec[0].rearrange("(c p) -> p c", p=128))
        # all q_proj rows as columns [p(h'), (b, c)] -> per-partition tanh bias;
        # one small gather up front keeps tiny descriptors out of the stream.
        qp_all = const.tile([128, b_loc * CB], f32)
        nc.sync.dma_start(
            out=qp_all, in_=qp.rearrange("b (c p) -> p (b c)", p=128)
        )

        reft_pair = {}
        for b in range(b_loc):
            u_sb = urow.tile([1, s_loc], f32)
            for st in range(st_per_batch):
                g = b * st_per_batch + st
                # load supertiles in pairs: one 1 MiB DMA, 8 KiB runs/partition
                if g % 2 == 0:
                    pair = inp.tile([128, 2 * KB * ST_ROWS], fin)
                    nc.sync.dma_start(
                        out=pair,
                        in_=reft[g // 2].rearrange("p g2 kb n -> p (g2 kb n)"),
                    )
                    reft_pair[g] = pair
                    reft_sb = pair[:, : KB * ST_ROWS]
                else:
                    reft_sb = reft_pair.pop(g - 1)[:, KB * ST_ROWS :]
                out_sb = outp.tile([128, CB * ST_ROWS], f32)
                u_ps = psu.tile([1, ST_ROWS], f32)
                for c in range(CB):
                    ps = psmm.tile([128, ST_ROWS], f32)
                    for kb in range(KB):
                        nc.tensor.matmul(
                            ps,
                            lhsT=wrefT_sb[
                                :, kb * H + c * 128 : kb * H + (c + 1) * 128
                            ],
                            rhs=reft_sb[:, kb * ST_ROWS : (kb + 1) * ST_ROWS],
                            start=(kb == 0),
                            stop=(kb == KB - 1),
                        )
                    nc.vector.tensor_copy(
                        out=out_sb[:, c * ST_ROWS : (c + 1) * ST_ROWS], in_=ps
                    )
                    th = tap.tile([128, ST_ROWS], f32r, tag="th")
                    nc.scalar.activation(
                        th,
                        ps,
                        mybir.ActivationFunctionType.Tanh,
                        bias=qp_all[:, b * CB + c : b * CB + c + 1],
                    )
                    nc.tensor.matmul(
                        u_ps,
                        lhsT=v_col[:, c : c + 1],
                        rhs=th,
                        start=(c == 0),
                        stop=(c == CB - 1),
                    )
                    if c % 2 == 1:  # store per chunk-pair: drains earlier
                        h0 = (c - 1) * ST_ROWS
                        h1 = (c + 1) * ST_ROWS
                        # alternate stores across both HWDGE rings
                        eng = nc.scalar if (g + c // 2) % 2 == 0 else nc.sync
                        eng.dma_start(
                            out=refprojT[g].rearrange("p c n -> p (c n)")[:, h0:h1],
                            in_=out_sb[:, h0:h1],
                        )
                nc.scalar.copy(
                    out=u_sb[:, st * ST_ROWS : (st + 1) * ST_ROWS], in_=u_ps
                )
            nc.scalar.dma_start(out=u[b : b + 1, :], in_=u_sb)

    nc.compile()
    return nc


def _get_program():
    key = (B_LOC, ST_PER_BATCH)
    if key not in _PROGRAM_CACHE:
        _PROGRAM_CACHE[key] = _build_program()
    return _PROGRAM_CACHE[key]


def _round_fp32r(x):
    """Round f32 to the fp32r-representable set (low 12 mantissa bits
    cleared, round-to-nearest) — bit-identical to walrus' fp32_to_fp32r."""
    b = x.view(np.uint32)
    rounded = (b + np.uint32(0x7FF) + ((b >> np.uint32(12)) & np.uint32(1))) & np.uint32(
        0xFFFFF000
    )
    return rounded.view(np.float32)


def kernel(q, ref, v, Wq, Wref):
    global LAST_RESULT
    import os
    from concourse.bass_utils import run_bass_kernel_spmd

    q = np.ascontiguousarray(np.asarray(q), dtype=np.float32)
    ref = np.ascontiguousarray(np.asarray(ref), dtype=np.float32)
    v = np.ascontiguousarray(np.asarray(v), dtype=np.float32)
    Wq = np.ascontiguousarray(np.asarray(Wq), dtype=np.float32)
    Wref = np.ascontiguousarray(np.asarray(Wref), dtype=np.float32)

    # Host-side prep (small): q projection, Wref transpose, ref supertile
    # transpose into partition-major layout [g, p, kb, n] so each partition's
    # DMA run on device is one contiguous 8 KiB.
    qp_full = np.ascontiguousarray(q @ Wq.T)                       # [B, H]
    wrefT = np.ascontiguousarray(
        Wref.T.reshape(KB, 128, H).transpose(1, 0, 2)
    )                                                              # [p, kb, h']
    reft_full = np.ascontiguousarray(
        ref.reshape(N_CORES * N_ST // 2, 2, ST_ROWS, KB, 128).transpose(0, 4, 1, 3, 2)
    )                                                              # [gp, p, g2, kb, n]
    if IN_DTYPE == "f16":
        wrefT = wrefT.astype(np.float16)
        reft_full = reft_full.astype(np.float16)
    else:
        wrefT = _round_fp32r(wrefT)
        reft_full = _round_fp32r(reft_full)

    vvec = _round_fp32r(v.reshape(1, H).copy())
    in_maps = [
        {
            "reft": reft_full[c * (N_ST // 2) : (c + 1) * (N_ST // 2)],
            "wrefT": wrefT,
            "qp": qp_full[c * B_LOC : (c + 1) * B_LOC],
            "vvec": vvec,
        }
        for c in range(N_CORES)
    ]

    nc = _get_program()
    res = run_bass_kernel_spmd(
        nc,
        in_maps,
        list(range(N_CORES)),
        tmpdir=os.environ.get("BASS_SPMD_TMPDIR"),
    )
    LAST_RESULT = res

    # refprojT per core: [g, p, c, n] with h' = c*128 + p, rows = g*512 + n.
    ref_proj = np.ascontiguousarray(
        np.stack([res.results[c]["refprojT"] for c in range(N_CORES)]).transpose(
            0, 1, 4, 3, 2
        )
    ).reshape(B, S, H)
    u = np.concatenate([res.results[c]["u"] for c in range(N_CORES)], axis=0)
    return u, ref_proj
